# revision 4
# baseline (speedup 1.0000x reference)
"""MetricSelfAttention on 8 TRN2 NeuronCores.

Batch-parallel SPMD: each core handles 2 of the 16 batches end-to-end
(no collectives). Host pre-transposes x / W1 / W2 / pre_metric so every
matmul operand lands in SBUF with its contraction dim on partitions.

Per core:
  phase 1: P = x @ W1.T + b1            (xT cached, W1T streamed once)
  phase M: M[n] = (L L^T)/sqrt(k), L = pre_metric*tril   (all heads)
  phase 2: per (b, n):  MP = M @ P;  S^T = MP^T @ P (tril-masked);
           OHT = P^T @ S^T  -> DRAM scratch
  phase 3: y = OH @ W2.T + b2           (OHT cached, W2T streamed once)
"""

import math
import sys

import numpy as np

try:
    import concourse.bass as bass
except ImportError:  # fresh grading dir: toolchain lives at fixed paths
    for p in ("/opt/trn_rl_repo", "/opt/pypackages"):
        if p not in sys.path:
            sys.path.insert(0, p)
    import concourse.bass as bass

import bass_rust as _bass_rust

import concourse.mybir as mybir
from concourse.bass_utils import run_bass_kernel_spmd
from concourse.tile import TileContext
from concourse.vector_clock import ScopedClock

F32 = mybir.dt.float32
P = 128
B, W, C, N = 16, 512, 4096, 8
NCORES = 8
BL = B // NCORES  # batches per core
T = BL * W  # tokens per core
KH = C // N  # per-head dim (== W)
SCALE = 1.0 / math.sqrt(KH)


class PatchedTileContext(TileContext):
    """This walrus build rejects instructions carrying >1 sync wait; the
    stock exit drain carries one wait per outstanding semaphore. Spread
    them across single-wait nops instead."""

    def _drain_and_barrier(self, tick_clock, wait_clock):
        carrier = self.nc.sync.nop(nofuse=True)
        wait_clock.add_sem_waits(
            carrier.ins, ScopedClock({None: tick_clock.global_clock})
        )
        si = carrier.ins.sync_info
        waits = list(si.on_wait) if si is not None else []
        if len(waits) > 1:
            si.on_wait = waits[:1]
            for w in waits[1:]:
                extra = self.nc.sync.nop(nofuse=True)
                extra.ins.sync_info = _bass_rust.SyncInfo(on_wait=[w], on_update=[])
        self.nc.sync.drain()

        self.nc.all_engine_barrier()
        popped = self.nc._tile_sem_poison_stack.pop()
        assert popped is self._sem_poison
        self.nc.clear_and_free_semaphores(list(self.sems.allocated().values()))
        self.nc.all_engine_barrier()


def spread_sync_waits(nc):
    """Hoist all-but-one sync wait of every instruction onto single-wait
    nops inserted just before it on the same engine (queues dispatch in
    order, so semantics are preserved)."""
    k = 0
    for fn in nc.m.functions:
        for bb in fn.blocks:
            out = []
            for inst in bb.instructions:
                si = inst.sync_info
                if si is not None and len(si.on_wait) > 1:
                    waits = list(si.on_wait)
                    for w in waits[:-1]:
                        nop = mybir.InstNoOp(name=f"waitnop-{k}", ins=[], outs=[])
                        k += 1
                        nop.engine = inst.engine
                        nop.sync_info = _bass_rust.SyncInfo(on_wait=[w], on_update=[])
                        out.append(nop)
                    si.on_wait = waits[-1:]
                out.append(inst)
            bb.instructions = out


def _build():
    nc = bass.Bass()
    xT = nc.dram_tensor("xT", [C, T], F32, kind="ExternalInput")
    W1T = nc.dram_tensor("W1T", [C, C], F32, kind="ExternalInput")
    b1 = nc.dram_tensor("b1", [C], F32, kind="ExternalInput")
    pmT = nc.dram_tensor("pmT", [N, W, W], F32, kind="ExternalInput")
    W2T = nc.dram_tensor("W2T", [C, C], F32, kind="ExternalInput")
    b2 = nc.dram_tensor("b2", [C], F32, kind="ExternalInput")
    y = nc.dram_tensor("y", [T, C], F32, kind="ExternalOutput")

    xT_r = xT.rearrange("(eo p) t -> p eo t", p=P)  # [128, 32, 1024]
    W1T_r = W1T.rearrange("(eo p) c -> p eo c", p=P)  # [128, 32, 4096]
    W2T_r = W2T.rearrange("(co p) m -> p co m", p=P)
    y_r = y.rearrange("(to p) m -> p to m", p=P)  # [128, 8, 4096]

    with PatchedTileContext(nc) as tc:
        with tc.tile_pool(name="dram", bufs=1, space="DRAM") as dram, \
             tc.tile_pool(name="const", bufs=1) as const:
            Pd = dram.tile([T, C], F32)
            OHT = dram.tile([BL, C, W], F32)
            Pd_r = Pd[:].rearrange("(to p) c -> p to c", p=P)  # [128, 8, 4096]

            # masks[vc][p, u] = 1.0 iff u >= vc*128 + p  (tril^T chunks)
            masks = const.tile([P, 4, W], F32)
            for vc in range(4):
                nc.gpsimd.memset(masks[:, vc, :], 1.0)
                nc.gpsimd.affine_select(
                    out=masks[:, vc, :],
                    in_=masks[:, vc, :],
                    compare_op=mybir.AluOpType.is_ge,
                    fill=0.0,
                    base=-vc * P,
                    pattern=[[1, W]],
                    channel_multiplier=-1,
                )

            # ---------------- phase 1: P = x @ W1.T + b1 ----------------
            CSL = 256  # W1T column-slab width
            with tc.tile_pool(name="xc", bufs=1) as xc_pool, \
                 tc.tile_pool(name="w1", bufs=2) as w1_pool, \
                 tc.tile_pool(name="bias1", bufs=2) as b1_pool, \
                 tc.tile_pool(name="p1out", bufs=3) as p1out, \
                 tc.tile_pool(name="psum1", bufs=4, space="PSUM") as psum1:
                xc = xc_pool.tile([P, 32, T], F32)
                for q in range(8):
                    nc.sync.dma_start(
                        xc[:, q * 4 : (q + 1) * 4, :], xT_r[:, q * 4 : (q + 1) * 4, :]
                    )
                for co in range(C // CSL):
                    w1s = w1_pool.tile([P, 32, CSL], F32)
                    for q in range(4):
                        nc.sync.dma_start(
                            w1s[:, q * 8 : (q + 1) * 8, :],
                            W1T_r[:, q * 8 : (q + 1) * 8, co * CSL : (co + 1) * CSL],
                        )
                    b1s = b1_pool.tile([P, CSL], F32)
                    nc.sync.dma_start(
                        b1s[:], b1[co * CSL : (co + 1) * CSL][None, :].to_broadcast((P, CSL))
                    )
                    for tch in range(8):
                        ps = psum1.tile([P, CSL], F32)
                        for e in range(32):
                            nc.tensor.matmul(
                                ps[:],
                                xc[:, e, tch * P : (tch + 1) * P],
                                w1s[:, e, :],
                                start=(e == 0),
                                stop=(e == 31),
                            )
                        po = p1out.tile([P, CSL], F32)
                        nc.vector.tensor_add(po[:], ps[:], b1s[:])
                        nc.sync.dma_start(
                            Pd_r[:, tch, co * CSL : (co + 1) * CSL], po[:]
                        )

            # ---------------- phase M + phase 2 ----------------
            with tc.tile_pool(name="msb", bufs=1) as m_pool:
                M_sb = m_pool.tile([P, N * 4, W], F32)
                with tc.tile_pool(name="pm", bufs=2) as pm_pool, \
                     tc.tile_pool(name="psumM", bufs=4, space="PSUM") as psumM:
                    for n in range(N):
                        lt = pm_pool.tile([P, 4, W], F32)
                        nc.sync.dma_start(
                            lt[:], pmT[n].rearrange("(vc p) u -> p vc u", p=P)
                        )
                        for vc in range(4):
                            nc.vector.tensor_mul(
                                lt[:, vc, :], lt[:, vc, :], masks[:, vc, :]
                            )
                        for uc in range(4):
                            ps = psumM.tile([P, W], F32)
                            for vc in range(4):
                                nc.tensor.matmul(
                                    ps[:],
                                    lt[:, vc, uc * P : (uc + 1) * P],
                                    lt[:, vc, :],
                                    start=(vc == 0),
                                    stop=(vc == 3),
                                )
                            nc.scalar.mul(M_sb[:, n * 4 + uc, :], ps[:], SCALE)

                with tc.tile_pool(name="ph", bufs=3) as ph_pool, \
                     tc.tile_pool(name="mp", bufs=2) as mp_pool, \
                     tc.tile_pool(name="st", bufs=2) as st_pool, \
                     tc.tile_pool(name="oh", bufs=4) as oh_pool, \
                     tc.tile_pool(name="psum2", bufs=4, space="PSUM") as psum2:
                    for b in range(BL):
                        for n in range(N):
                            ph = ph_pool.tile([P, 4, W], F32)
                            nc.sync.dma_start(
                                ph[:],
                                Pd_r[:, b * 4 : (b + 1) * 4, n * KH : (n + 1) * KH],
                            )
                            mp = mp_pool.tile([P, 4, W], F32)
                            for uc in range(4):
                                ps = psum2.tile([P, W], F32)
                                for vc in range(4):
                                    nc.tensor.matmul(
                                        ps[:],
                                        M_sb[:, n * 4 + vc, uc * P : (uc + 1) * P],
                                        ph[:, vc, :],
                                        start=(vc == 0),
                                        stop=(vc == 3),
                                    )
                                nc.scalar.copy(mp[:, uc, :], ps[:])
                            st = st_pool.tile([P, 4, W], F32)
                            for jc in range(4):
                                ps = psum2.tile([P, W], F32)
                                for uc in range(4):
                                    nc.tensor.matmul(
                                        ps[:],
                                        mp[:, uc, jc * P : (jc + 1) * P],
                                        ph[:, uc, :],
                                        start=(uc == 0),
                                        stop=(uc == 3),
                                    )
                                nc.vector.tensor_mul(
                                    st[:, jc, :], ps[:], masks[:, jc, :]
                                )
                            for lc in range(4):
                                ps = psum2.tile([P, W], F32)
                                for jc in range(4):
                                    nc.tensor.matmul(
                                        ps[:],
                                        ph[:, jc, lc * P : (lc + 1) * P],
                                        st[:, jc, :],
                                        start=(jc == 0),
                                        stop=(jc == 3),
                                    )
                                oh = oh_pool.tile([P, W], F32)
                                nc.vector.tensor_copy(oh[:], ps[:])
                                nc.sync.dma_start(
                                    OHT[
                                        b,
                                        n * KH + lc * P : n * KH + (lc + 1) * P,
                                        :,
                                    ],
                                    oh[:],
                                )

            # ---------------- phase 3: y = OH @ W2.T + b2 ----------------
            MSL = 256  # W2T column-slab width
            with tc.tile_pool(name="a3", bufs=1) as a_pool, \
                 tc.tile_pool(name="w2", bufs=2) as w2_pool, \
                 tc.tile_pool(name="bias2", bufs=2) as b2_pool, \
                 tc.tile_pool(name="yout", bufs=3) as y_pool, \
                 tc.tile_pool(name="psum3", bufs=4, space="PSUM") as psum3:
                a = a_pool.tile([P, 32, T], F32)  # [c-chunk, t = b*512 + i]
                for b in range(BL):
                    oht_r = OHT[b].rearrange("(co p) i -> p co i", p=P)
                    for q in range(8):
                        nc.sync.dma_start(
                            a[:, q * 4 : (q + 1) * 4, b * W : (b + 1) * W],
                            oht_r[:, q * 4 : (q + 1) * 4, :],
                        )
                for mo in range(C // MSL):
                    w2s = w2_pool.tile([P, 32, MSL], F32)
                    for q in range(4):
                        nc.sync.dma_start(
                            w2s[:, q * 8 : (q + 1) * 8, :],
                            W2T_r[:, q * 8 : (q + 1) * 8, mo * MSL : (mo + 1) * MSL],
                        )
                    b2s = b2_pool.tile([P, MSL], F32)
                    nc.sync.dma_start(
                        b2s[:], b2[mo * MSL : (mo + 1) * MSL][None, :].to_broadcast((P, MSL))
                    )
                    for tch in range(8):
                        ps = psum3.tile([P, MSL], F32)
                        for cc in range(32):
                            nc.tensor.matmul(
                                ps[:],
                                a[:, cc, tch * P : (tch + 1) * P],
                                w2s[:, cc, :],
                                start=(cc == 0),
                                stop=(cc == 31),
                            )
                        yo = y_pool.tile([P, MSL], F32)
                        nc.vector.tensor_add(yo[:], ps[:], b2s[:])
                        nc.sync.dma_start(
                            y_r[:, tch, mo * MSL : (mo + 1) * MSL], yo[:]
                        )

    spread_sync_waits(nc)
    return nc


_NC_CACHE = None
_last_in_maps = None


def kernel(**inputs: np.ndarray) -> np.ndarray:
    global _NC_CACHE, _last_in_maps
    x = np.asarray(inputs["x"], dtype=np.float32)
    W1 = np.asarray(inputs["W1"], dtype=np.float32)
    b1 = np.asarray(inputs["b1"], dtype=np.float32)
    pre_metric = np.asarray(inputs["pre_metric"], dtype=np.float32)
    W2 = np.asarray(inputs["W2"], dtype=np.float32)
    b2 = np.asarray(inputs["b2"], dtype=np.float32)

    W1T = np.ascontiguousarray(W1.T)
    W2T = np.ascontiguousarray(W2.T)
    pmT = np.ascontiguousarray(pre_metric.transpose(0, 2, 1))
    xr = x.reshape(NCORES, T, C)

    in_maps = []
    for i in range(NCORES):
        in_maps.append(
            {
                "xT": np.ascontiguousarray(xr[i].T),
                "W1T": W1T,
                "b1": b1,
                "pmT": pmT,
                "W2T": W2T,
                "b2": b2,
            }
        )

    _last_in_maps = in_maps
    if _NC_CACHE is None:
        _NC_CACHE = _build()
    res = run_bass_kernel_spmd(_NC_CACHE, in_maps, list(range(NCORES)))
    out = np.concatenate(
        [res.results[i]["y"].reshape(BL, W, C) for i in range(NCORES)], axis=0
    )
    return out.astype(np.float32)


if __name__ == "__main__":
    rng = np.random.default_rng(0)
    ins = {
        "x": rng.standard_normal((B, W, C), dtype=np.float32),
        "W1": (rng.standard_normal((C, C), dtype=np.float32) * 0.02),
        "b1": (rng.standard_normal((C,), dtype=np.float32) * 0.02),
        "pre_metric": (rng.standard_normal((N, W, W), dtype=np.float32) * 0.02),
        "W2": (rng.standard_normal((C, C), dtype=np.float32) * 0.02),
        "b2": (rng.standard_normal((C,), dtype=np.float32) * 0.02),
    }
    out = kernel(**ins)
    print("kernel output shape:", out.shape, out.dtype)


# revision 5
# speedup vs baseline: 3.7842x; 3.7842x over previous
"""MetricSelfAttention on 8 TRN2 NeuronCores.

Batch-parallel SPMD: each core handles 2 of the 16 batches end-to-end
(no collectives). Host pre-transposes x / W1 / W2 / pre_metric so every
matmul operand lands in SBUF with its contraction dim on partitions.

Per core:
  phase 1: P = x @ W1.T + b1            (xT cached, W1T streamed once)
  phase M: M[n] = (L L^T)/sqrt(k), L = pre_metric*tril   (all heads)
  phase 2: per (b, n):  MP = M @ P;  S^T = MP^T @ P (tril-masked);
           OHT = P^T @ S^T  -> DRAM scratch
  phase 3: y = OH @ W2.T + b2           (OHT cached, W2T streamed once)

Matmul operands are bf16 (PSUM accumulation stays fp32); fp32 streams
at 4 cycles/column on the PE (2 passes x 2 cycles for 4-byte operands),
so bf16 is 4x tensor-engine throughput. Measured end-to-end relative
error vs the fp32 reference is ~5e-3.
"""

import math
import sys

import numpy as np

try:
    import concourse.bass as bass
except ImportError:  # fresh grading dir: toolchain lives at fixed paths
    for p in ("/opt/trn_rl_repo", "/opt/pypackages"):
        if p not in sys.path:
            sys.path.insert(0, p)
    import concourse.bass as bass

import bass_rust as _bass_rust
import ml_dtypes

import concourse.mybir as mybir
from concourse.bass_utils import run_bass_kernel_spmd
from concourse.tile import TileContext
from concourse.vector_clock import ScopedClock

F32 = mybir.dt.float32
BF16 = mybir.dt.bfloat16
NP_BF16 = ml_dtypes.bfloat16
P = 128
B, W, C, N = 16, 512, 4096, 8
NCORES = 8
BL = B // NCORES  # batches per core
T = BL * W  # tokens per core
KH = C // N  # per-head dim (== W)
SCALE = 1.0 / math.sqrt(KH)


class PatchedTileContext(TileContext):
    """This walrus build rejects instructions carrying >1 sync wait; the
    stock exit drain carries one wait per outstanding semaphore. Spread
    them across single-wait nops instead."""

    def _drain_and_barrier(self, tick_clock, wait_clock):
        carrier = self.nc.sync.nop(nofuse=True)
        wait_clock.add_sem_waits(
            carrier.ins, ScopedClock({None: tick_clock.global_clock})
        )
        si = carrier.ins.sync_info
        waits = list(si.on_wait) if si is not None else []
        if len(waits) > 1:
            si.on_wait = waits[:1]
            for w in waits[1:]:
                extra = self.nc.sync.nop(nofuse=True)
                extra.ins.sync_info = _bass_rust.SyncInfo(on_wait=[w], on_update=[])
        self.nc.sync.drain()

        self.nc.all_engine_barrier()
        popped = self.nc._tile_sem_poison_stack.pop()
        assert popped is self._sem_poison
        self.nc.clear_and_free_semaphores(list(self.sems.allocated().values()))
        self.nc.all_engine_barrier()


def spread_sync_waits(nc):
    """Hoist all-but-one sync wait of every instruction onto single-wait
    nops inserted just before it on the same engine (queues dispatch in
    order, so semantics are preserved)."""
    k = 0
    for fn in nc.m.functions:
        for bb in fn.blocks:
            out = []
            for inst in bb.instructions:
                si = inst.sync_info
                if si is not None and len(si.on_wait) > 1:
                    waits = list(si.on_wait)
                    for w in waits[:-1]:
                        nop = mybir.InstNoOp(name=f"waitnop-{k}", ins=[], outs=[])
                        k += 1
                        nop.engine = inst.engine
                        nop.sync_info = _bass_rust.SyncInfo(on_wait=[w], on_update=[])
                        out.append(nop)
                    si.on_wait = waits[-1:]
                out.append(inst)
            bb.instructions = out


def _build():
    nc = bass.Bass()
    xT = nc.dram_tensor("xT", [C, T], BF16, kind="ExternalInput")
    W1T = nc.dram_tensor("W1T", [C, C], BF16, kind="ExternalInput")
    b1 = nc.dram_tensor("b1", [C], F32, kind="ExternalInput")
    pmT = nc.dram_tensor("pmT", [N, W, W], BF16, kind="ExternalInput")
    W2T = nc.dram_tensor("W2T", [C, C], BF16, kind="ExternalInput")
    b2 = nc.dram_tensor("b2", [C], F32, kind="ExternalInput")
    y = nc.dram_tensor("y", [T, C], F32, kind="ExternalOutput")

    xT_r = xT.rearrange("(eo p) t -> p eo t", p=P)  # [128, 32, 1024]
    W1T_r = W1T.rearrange("(eo p) c -> p eo c", p=P)  # [128, 32, 4096]
    W2T_r = W2T.rearrange("(co p) m -> p co m", p=P)
    y_r = y.rearrange("(to p) m -> p to m", p=P)  # [128, 8, 4096]

    with PatchedTileContext(nc) as tc:
        with tc.tile_pool(name="dram", bufs=1, space="DRAM") as dram, \
             tc.tile_pool(name="const", bufs=1) as const:
            Pd = dram.tile([T, C], BF16)
            OHT = dram.tile([BL, C, W], BF16)
            Pd_r = Pd[:].rearrange("(to p) c -> p to c", p=P)  # [128, 8, 4096]

            # masks[vc][p, u] = 1.0 iff u >= vc*128 + p  (tril^T chunks)
            maskf = const.tile([P, 4, W], F32)
            maskb = const.tile([P, 4, W], BF16)
            for vc in range(4):
                nc.gpsimd.memset(maskf[:, vc, :], 1.0)
                nc.gpsimd.affine_select(
                    out=maskf[:, vc, :],
                    in_=maskf[:, vc, :],
                    compare_op=mybir.AluOpType.is_ge,
                    fill=0.0,
                    base=-vc * P,
                    pattern=[[1, W]],
                    channel_multiplier=-1,
                )
                nc.vector.tensor_copy(maskb[:, vc, :], maskf[:, vc, :])

            # ---------------- phase 1: P = x @ W1.T + b1 ----------------
            CSL = 512  # W1T column-slab width
            with tc.tile_pool(name="xc", bufs=1) as xc_pool, \
                 tc.tile_pool(name="w1", bufs=2) as w1_pool, \
                 tc.tile_pool(name="bias1", bufs=2) as b1_pool, \
                 tc.tile_pool(name="p1out", bufs=4) as p1out, \
                 tc.tile_pool(name="psum1", bufs=4, space="PSUM") as psum1:
                xc = xc_pool.tile([P, 32, T], BF16)
                for q in range(8):
                    nc.sync.dma_start(
                        xc[:, q * 4 : (q + 1) * 4, :], xT_r[:, q * 4 : (q + 1) * 4, :]
                    )
                for co in range(C // CSL):
                    w1s = w1_pool.tile([P, 32, CSL], BF16)
                    for q in range(4):
                        nc.sync.dma_start(
                            w1s[:, q * 8 : (q + 1) * 8, :],
                            W1T_r[:, q * 8 : (q + 1) * 8, co * CSL : (co + 1) * CSL],
                        )
                    b1s = b1_pool.tile([P, CSL], F32)
                    nc.sync.dma_start(
                        b1s[:],
                        b1[co * CSL : (co + 1) * CSL][None, :].to_broadcast((P, CSL)),
                    )
                    for tch in range(8):
                        ps = psum1.tile([P, CSL], F32)
                        for e in range(32):
                            nc.tensor.matmul(
                                ps[:],
                                xc[:, e, tch * P : (tch + 1) * P],
                                w1s[:, e, :],
                                start=(e == 0),
                                stop=(e == 31),
                            )
                        po = p1out.tile([P, CSL], BF16)
                        nc.vector.tensor_add(po[:], ps[:], b1s[:])
                        nc.sync.dma_start(
                            Pd_r[:, tch, co * CSL : (co + 1) * CSL], po[:]
                        )

            # ---------------- phase M + phase 2 ----------------
            with tc.tile_pool(name="msb", bufs=1) as m_pool:
                M_sb = m_pool.tile([P, N * 4, W], BF16)
                with tc.tile_pool(name="pm", bufs=2) as pm_pool, \
                     tc.tile_pool(name="psumM", bufs=4, space="PSUM") as psumM:
                    for n in range(N):
                        lt = pm_pool.tile([P, 4, W], BF16)
                        nc.sync.dma_start(
                            lt[:], pmT[n].rearrange("(vc p) u -> p vc u", p=P)
                        )
                        for vc in range(4):
                            nc.vector.tensor_mul(
                                lt[:, vc, :], lt[:, vc, :], maskb[:, vc, :]
                            )
                        for uc in range(4):
                            ps = psumM.tile([P, W], F32)
                            for vc in range(4):
                                nc.tensor.matmul(
                                    ps[:],
                                    lt[:, vc, uc * P : (uc + 1) * P],
                                    lt[:, vc, :],
                                    start=(vc == 0),
                                    stop=(vc == 3),
                                )
                            nc.scalar.mul(M_sb[:, n * 4 + uc, :], ps[:], SCALE)

                with tc.tile_pool(name="ph", bufs=3) as ph_pool, \
                     tc.tile_pool(name="mp", bufs=2) as mp_pool, \
                     tc.tile_pool(name="st", bufs=2) as st_pool, \
                     tc.tile_pool(name="oh", bufs=4) as oh_pool, \
                     tc.tile_pool(name="psum2", bufs=4, space="PSUM") as psum2:
                    for b in range(BL):
                        for n in range(N):
                            ph = ph_pool.tile([P, 4, W], BF16)
                            nc.sync.dma_start(
                                ph[:],
                                Pd_r[:, b * 4 : (b + 1) * 4, n * KH : (n + 1) * KH],
                            )
                            mp = mp_pool.tile([P, 4, W], BF16)
                            for uc in range(4):
                                ps = psum2.tile([P, W], F32)
                                for vc in range(4):
                                    nc.tensor.matmul(
                                        ps[:],
                                        M_sb[:, n * 4 + vc, uc * P : (uc + 1) * P],
                                        ph[:, vc, :],
                                        start=(vc == 0),
                                        stop=(vc == 3),
                                    )
                                nc.scalar.copy(mp[:, uc, :], ps[:])
                            st = st_pool.tile([P, 4, W], BF16)
                            for jc in range(4):
                                ps = psum2.tile([P, W], F32)
                                for uc in range(4):
                                    nc.tensor.matmul(
                                        ps[:],
                                        mp[:, uc, jc * P : (jc + 1) * P],
                                        ph[:, uc, :],
                                        start=(uc == 0),
                                        stop=(uc == 3),
                                    )
                                nc.vector.tensor_mul(
                                    st[:, jc, :], ps[:], maskf[:, jc, :]
                                )
                            for lc in range(4):
                                ps = psum2.tile([P, W], F32)
                                for jc in range(4):
                                    nc.tensor.matmul(
                                        ps[:],
                                        ph[:, jc, lc * P : (lc + 1) * P],
                                        st[:, jc, :],
                                        start=(jc == 0),
                                        stop=(jc == 3),
                                    )
                                oh = oh_pool.tile([P, W], BF16)
                                nc.vector.tensor_copy(oh[:], ps[:])
                                nc.sync.dma_start(
                                    OHT[
                                        b,
                                        n * KH + lc * P : n * KH + (lc + 1) * P,
                                        :,
                                    ],
                                    oh[:],
                                )

            # ---------------- phase 3: y = OH @ W2.T + b2 ----------------
            MSL = 512  # W2T column-slab width
            with tc.tile_pool(name="a3", bufs=1) as a_pool, \
                 tc.tile_pool(name="w2", bufs=2) as w2_pool, \
                 tc.tile_pool(name="bias2", bufs=2) as b2_pool, \
                 tc.tile_pool(name="yout", bufs=4) as y_pool, \
                 tc.tile_pool(name="psum3", bufs=4, space="PSUM") as psum3:
                a = a_pool.tile([P, 32, T], BF16)  # [c-chunk, t = b*512 + i]
                for b in range(BL):
                    oht_r = OHT[b].rearrange("(co p) i -> p co i", p=P)
                    for q in range(8):
                        nc.sync.dma_start(
                            a[:, q * 4 : (q + 1) * 4, b * W : (b + 1) * W],
                            oht_r[:, q * 4 : (q + 1) * 4, :],
                        )
                for mo in range(C // MSL):
                    w2s = w2_pool.tile([P, 32, MSL], BF16)
                    for q in range(4):
                        nc.sync.dma_start(
                            w2s[:, q * 8 : (q + 1) * 8, :],
                            W2T_r[:, q * 8 : (q + 1) * 8, mo * MSL : (mo + 1) * MSL],
                        )
                    b2s = b2_pool.tile([P, MSL], F32)
                    nc.sync.dma_start(
                        b2s[:],
                        b2[mo * MSL : (mo + 1) * MSL][None, :].to_broadcast((P, MSL)),
                    )
                    for tch in range(8):
                        ps = psum3.tile([P, MSL], F32)
                        for cc in range(32):
                            nc.tensor.matmul(
                                ps[:],
                                a[:, cc, tch * P : (tch + 1) * P],
                                w2s[:, cc, :],
                                start=(cc == 0),
                                stop=(cc == 31),
                            )
                        yo = y_pool.tile([P, MSL], F32)
                        nc.vector.tensor_add(yo[:], ps[:], b2s[:])
                        nc.sync.dma_start(
                            y_r[:, tch, mo * MSL : (mo + 1) * MSL], yo[:]
                        )

    spread_sync_waits(nc)
    return nc


_NC_CACHE = None
_last_in_maps = None


def kernel(**inputs: np.ndarray) -> np.ndarray:
    global _NC_CACHE, _last_in_maps
    x = np.asarray(inputs["x"], dtype=np.float32)
    W1 = np.asarray(inputs["W1"], dtype=np.float32)
    b1 = np.asarray(inputs["b1"], dtype=np.float32)
    pre_metric = np.asarray(inputs["pre_metric"], dtype=np.float32)
    W2 = np.asarray(inputs["W2"], dtype=np.float32)
    b2 = np.asarray(inputs["b2"], dtype=np.float32)

    W1T = np.ascontiguousarray(W1.T).astype(NP_BF16)
    W2T = np.ascontiguousarray(W2.T).astype(NP_BF16)
    pmT = np.ascontiguousarray(pre_metric.transpose(0, 2, 1)).astype(NP_BF16)
    xr = x.reshape(NCORES, T, C)

    in_maps = []
    for i in range(NCORES):
        in_maps.append(
            {
                "xT": np.ascontiguousarray(xr[i].T).astype(NP_BF16),
                "W1T": W1T,
                "b1": b1,
                "pmT": pmT,
                "W2T": W2T,
                "b2": b2,
            }
        )

    _last_in_maps = in_maps
    if _NC_CACHE is None:
        _NC_CACHE = _build()
    res = run_bass_kernel_spmd(_NC_CACHE, in_maps, list(range(NCORES)))
    out = np.concatenate(
        [res.results[i]["y"].reshape(BL, W, C) for i in range(NCORES)], axis=0
    )
    return out.astype(np.float32)


if __name__ == "__main__":
    rng = np.random.default_rng(0)
    ins = {
        "x": rng.standard_normal((B, W, C), dtype=np.float32),
        "W1": (rng.standard_normal((C, C), dtype=np.float32) * 0.02),
        "b1": (rng.standard_normal((C,), dtype=np.float32) * 0.02),
        "pre_metric": (rng.standard_normal((N, W, W), dtype=np.float32) * 0.02),
        "W2": (rng.standard_normal((C, C), dtype=np.float32) * 0.02),
        "b2": (rng.standard_normal((C,), dtype=np.float32) * 0.02),
    }
    out = kernel(**ins)
    print("kernel output shape:", out.shape, out.dtype)
